# revision 7
# baseline (speedup 1.0000x reference)
"""AttentionalPropagation kernel for Trainium2 (Bass/Tile), 8-core SPMD.

x: [B=64, C=512, L=4096] f32.  Per location l: self-attention over the B axis
(q=k=v, head dim C), out = x + msg.  Sharded over L: each core takes LS=512
locations; the host repacks its slice to bf16 in a per-block DMA-friendly
layout [blk, c(128), ci(4), b(64), l(64)] so every DMA moves 32KB-contiguous
runs per partition (full-bandwidth descriptors), and unpacks the same layout
on the way back.

Per-core dataflow (pairs of 2 locations, groups of 8 pairs):
  - mm1: scores compact [128=(j,b), 64=b'] psum, 8 matmuls of N=64 (K=c
    chunks of 128, two locations stacked on output partitions)
  - exp on ACT with accum_out giving rowsums S per partition; batched
    reciprocal (DVE) -> inv bf16 [128, 8] per group
  - inv transposed via one PE matmul vs identity -> invT [8, 128];
    per pair gpsimd partition_broadcast -> invrep [128, 128] sbuf
  - P2 = E * invrep on diag blocks only into persistent zero-padded
    [128,128] bf16 tiles (block-diagonal normalized-transposed probs)
  - q_b: PE transpose of the x tile pair-slice -> bf16 psum, copied to
    sbuf (DVE/ACT, split by schedule)
  - mm2: U[c, (j,b)] = qb^T-contracted with P2 (4 chunk matmuls N=128);
    by symmetry of E this is msgT with per-column normalization already
    applied; residual x is folded in as one extra identity-matmul
    accumulate per chunk for pairs whose combine runs on ACT
  - combine: U (+x) written into the x tile in place (DVE tensor_add or
    ACT copy), then one DMA per block writes the finished tile out
"""

import numpy as np

B, C, L_FULL, N_CORES = 64, 512, 4096, 8
LS = L_FULL // N_CORES  # 512 locations per core
LB = 64                 # locations per block
N_BLK = LS // LB        # 8
N_PAIR = LB // 2        # 32 pairs per block
GRP = 8                 # pairs per group (reciprocal batch)
N_GRP = N_PAIR // GRP   # 4 groups per block
CCH = C // 128          # 4 c-chunks
SCALE = 1.0 / float(C) ** 0.5

# engine schedule tuning (per 2-pair move unit, global counter)
COMB_ACT_NUM, COMB_ACT_DEN = 7, 20   # fraction of combines on ACT (+PE residual)
QB_ACT_NUM, QB_ACT_DEN = 1, 4        # fraction of qb copies on ACT


def build_nc():
    from contextlib import ExitStack

    import concourse.bass as bass
    import concourse.mybir as mybir
    from concourse.masks import make_identity
    from concourse.tile import TileContext

    f32 = mybir.dt.float32
    bf16 = mybir.dt.bfloat16
    AF = mybir.ActivationFunctionType

    nc = bass.Bass()
    x = nc.dram_tensor("x", [N_BLK, 128, CCH, B, LB], bf16, kind="ExternalInput")
    y = nc.dram_tensor("y", [N_BLK, 128, CCH, B, LB], bf16, kind="ExternalOutput")

    with ExitStack() as ctx:
        tc = ctx.enter_context(TileContext(nc))
        const = ctx.enter_context(tc.tile_pool(name="const", bufs=1))
        xt_pool = ctx.enter_context(tc.tile_pool(name="xt", bufs=3))
        sm = ctx.enter_context(tc.tile_pool(name="sm", bufs=2))
        ps = ctx.enter_context(tc.tile_pool(name="ps", bufs=2, space="PSUM"))

        ident = const.tile([128, 128], bf16, name="ident", tag="ident")
        make_identity(nc, ident)
        # persistent block-diagonal prob tiles: off-diagonal zeroed once
        p2t = []
        for i in range(4):
            t = const.tile([128, 128], bf16, name=f"p2_{i}", tag=f"p2_{i}")
            nc.vector.memset(t, 0.0)
            p2t.append(t)

        xt_tiles = {}
        grp_state = {}  # g -> dict with e tiles, qbs tiles, invTs, block
        qctr = [0]      # global 2-pair counter (combine schedule)
        qbctr = [0]     # global 2-pair counter (qb copy schedule)
        p2ctr = [0]

        def emit_A_pair(g, gp, st):
            b = g // N_GRP
            xt = xt_tiles[b]
            p = (g % N_GRP) * GRP + gp  # pair index within block
            ps_s = st["ps_s"]
            for j in (0, 1):
                for ci in range(CCH):
                    nc.tensor.matmul(
                        ps_s[j * 64 : (j + 1) * 64, gp * 64 : (gp + 1) * 64],
                        xt[:, ci, :, 2 * p + j],
                        xt[:, ci, :, 2 * p + j],
                        start=(ci == 0),
                        stop=(ci == CCH - 1),
                    )
            e = sm.tile([128, 64], bf16, name=f"e{g}_{gp}", tag="e", bufs=18)
            nc.scalar.activation(
                e,
                ps_s[:, gp * 64 : (gp + 1) * 64],
                AF.Exp,
                scale=SCALE,
                accum_out=st["invB"][:, gp : gp + 1],
            )
            st["e"].append(e)

            slot = gp % 2
            if slot == 0:
                st["qbps"] = ps.tile(
                    [128, 1024], bf16, name=f"qbp{g}_{gp}", tag="qbps", bufs=2
                )
            qbps = st["qbps"]
            for ci in range(CCH):
                nc.tensor.transpose(
                    qbps[:, slot * 512 + ci * 128 : slot * 512 + (ci + 1) * 128],
                    xt[:, ci, :, 2 * p : 2 * p + 2].rearrange("c b l -> c l b"),
                    ident,
                )
            if slot == 1:
                qbs = sm.tile(
                    [128, 1024], bf16, name=f"qbs{g}_{gp}", tag="qbs", bufs=8
                )
                on_act = (qbctr[0] * QB_ACT_NUM) % QB_ACT_DEN < QB_ACT_NUM
                qbctr[0] += 1
                if on_act:
                    nc.scalar.copy(qbs, qbps)
                else:
                    nc.vector.tensor_copy(qbs, qbps)
                st["qbs"].append(qbs)

        def emit_B(g):
            st = grp_state[g]
            invb = sm.tile([128, GRP], bf16, name=f"invb{g}", tag="invb", bufs=2)
            with nc.allow_low_precision(reason="softmax norm tolerates bf16"):
                nc.vector.reciprocal(invb, st["invB"])
            ps_t = ps.tile([GRP, 128], f32, name=f"it{g}", tag="s", bufs=2)
            nc.tensor.matmul(ps_t, invb, ident, start=True, stop=True)
            invTs = sm.tile([GRP, 128], bf16, name=f"iT{g}", tag="invTs", bufs=2)
            nc.vector.tensor_copy(invTs, ps_t)
            st["invTs"] = invTs

        def emit_C_pair(g, gp):
            st = grp_state[g]
            b = g // N_GRP
            xt = xt_tiles[b]
            invTs = st["invTs"]
            p = (g % N_GRP) * GRP + gp
            invrep = sm.tile(
                [128, 128], bf16, name=f"ir{g}_{gp}", tag="invrep", bufs=3
            )
            nc.gpsimd.partition_broadcast(invrep, invTs[gp : gp + 1, :])
            e = st["e"][gp]
            P2 = p2t[p2ctr[0] % 4]
            p2ctr[0] += 1
            nc.gpsimd.tensor_mul(P2[0:64, 0:64], e[0:64, :], invrep[0:64, 0:64])
            nc.gpsimd.tensor_mul(
                P2[64:128, 64:128], e[64:128, :], invrep[64:128, 64:128]
            )

            slot = gp % 2
            if slot == 0:
                st["U"] = ps.tile(
                    [128, 1024], f32, name=f"u{g}_{gp}", tag="u", bufs=2
                )
                st["use_act"] = (
                    qctr[0] * COMB_ACT_NUM
                ) % COMB_ACT_DEN < COMB_ACT_NUM
                qctr[0] += 1
            U, use_act = st["U"], st["use_act"]
            qbs = st["qbs"][gp // 2]
            for ci in range(CCH):
                sl = slice(slot * 512 + ci * 128, slot * 512 + (ci + 1) * 128)
                nc.tensor.matmul(
                    U[:, sl], qbs[:, sl], P2, start=True, stop=not use_act
                )
                if use_act:
                    # residual: U += x^T (same layout), so ACT combine is a copy
                    nc.tensor.matmul(
                        U[:, sl], qbs[:, sl], ident, start=False, stop=True
                    )
            if slot == 1:
                q4 = p // 2  # 2-pair index within block -> l = 4*q4 .. 4*q4+3
                dst = xt[:, :, :, 4 * q4 : 4 * q4 + 4].rearrange(
                    "c ci b (s j) -> c ci b s j", s=2
                )
                src = U.rearrange("c (s ci j b) -> c ci b s j", s=2, ci=CCH, j=2)
                if use_act:
                    nc.scalar.copy(dst, src)
                else:
                    nc.vector.tensor_add(dst, src, dst)

        def emit_group(g):
            # A(g) interleaved pairwise with C(g-1), then B(g), then tail of C
            st = {
                "e": [],
                "qbs": [],
                "invB": sm.tile([128, GRP], f32, name=f"invB{g}", tag="invB", bufs=2),
                "ps_s": ps.tile([128, 512], f32, name=f"s{g}", tag="s", bufs=2),
            }
            grp_state[g] = st
            for gp in range(GRP):
                emit_A_pair(g, gp, st)
                if g >= 1:
                    emit_C_pair(g - 1, gp)
            emit_B(g)
            if g >= 1:
                grp_state.pop(g - 1)

        G_TOT = N_BLK * N_GRP
        for g in range(G_TOT):
            b = g // N_GRP
            if g % N_GRP == 0:
                xt = xt_pool.tile([128, CCH, B, LB], bf16, name=f"xt{b}", tag="xt")
                nc.sync.dma_start(out=xt, in_=x[b])
                xt_tiles[b] = xt
            emit_group(g)
            if g >= 1 and (g - 1) % N_GRP == N_GRP - 1:
                bb = (g - 1) // N_GRP
                nc.sync.dma_start(out=y[bb], in_=xt_tiles[bb])
        for gp in range(GRP):
            emit_C_pair(G_TOT - 1, gp)
        nc.sync.dma_start(out=y[N_BLK - 1], in_=xt_tiles[N_BLK - 1])

    _hoist_extra_waits(nc)
    return nc


def _hoist_extra_waits(nc):
    """The 64B instruction encodings have room for only one embedded
    sem-wait, but Tile sometimes emits 2+ (foreign engine + self).  Splice
    same-engine NoOps (one wait each) before such instructions; the
    instruction keeps its last wait plus its sem updates."""
    import concourse.mybir as mybir

    n_fixed = 0
    for f in nc.m.functions:
        for blk in f.blocks:
            new_insts = []
            for inst in blk.instructions:
                si = inst.sync_info
                if si is not None and len(si.on_wait) > 1:
                    waits = list(si.on_wait)
                    for wi, w in enumerate(waits[:-1]):
                        nop = mybir.InstNoOp(
                            name=f"{inst.name}-wsp{wi}", ins=[], outs=[]
                        )
                        nop.engine = inst.engine
                        nop.sync_info = mybir.SyncInfo(on_wait=[w], on_update=[])
                        new_insts.append(nop)
                    inst.sync_info = mybir.SyncInfo(
                        on_wait=[waits[-1]], on_update=list(si.on_update)
                    )
                    n_fixed += 1
                new_insts.append(inst)
            if n_fixed:
                try:
                    blk.instructions = new_insts
                except Exception:
                    blk.instructions.clear()
                    blk.instructions.extend(new_insts)
    return n_fixed


_NC_CACHE = {}


def _pack(xc: np.ndarray):
    """[B, C, LS] f32 -> [N_BLK, 128, CCH, B, LB] bf16 (contiguous)."""
    import ml_dtypes

    xr = xc.reshape(B, CCH, 128, N_BLK, LB).transpose(3, 2, 1, 0, 4)
    return np.ascontiguousarray(xr).astype(ml_dtypes.bfloat16)


def _unpack(yb: np.ndarray):
    """[N_BLK, 128, CCH, B, LB] bf16 -> [B, C, LS] f32."""
    return (
        yb.astype(np.float32).transpose(3, 2, 1, 0, 4).reshape(B, C, LS)
    )


def kernel(x: np.ndarray) -> np.ndarray:
    from concourse.bass_utils import run_bass_kernel_spmd

    assert x.shape == (B, C, L_FULL) and x.dtype == np.float32
    if "nc" not in _NC_CACHE:
        _NC_CACHE["nc"] = build_nc()
    nc = _NC_CACHE["nc"]

    in_maps = [
        {"x": _pack(x[:, :, i * LS : (i + 1) * LS])} for i in range(N_CORES)
    ]
    res = run_bass_kernel_spmd(nc, in_maps, core_ids=list(range(N_CORES)))
    out = np.concatenate(
        [_unpack(res.results[i]["y"]) for i in range(N_CORES)], axis=2
    )
    return out


# revision 14
# speedup vs baseline: 1.5032x; 1.5032x over previous
"""AttentionalPropagation kernel for Trainium2 (Bass/Tile), 8-core SPMD.

x: [B=64, C=512, L=4096] f32.  Per location l: self-attention over the B axis
(q=k=v, head dim C), out = x + msg.  Sharded over L: each core takes LS=512
locations; the host repacks its slice to bf16 in a per-block DMA-friendly
layout [blk, c(128), ci(4), b(64), l(64)] so every DMA moves 32KB-contiguous
runs per partition (full-bandwidth descriptors), and unpacks the same layout
on the way back.

Per-core dataflow (pairs of 2 locations, groups of 8 pairs):
  - mm1: scores compact [128=(j,b), 64=b'] psum, 8 matmuls of N=64 (K=c
    chunks of 128, two locations stacked on output partitions)
  - exp on ACT with accum_out giving rowsums S per partition; batched
    reciprocal (DVE) -> inv bf16 [128, 8] per group
  - inv transposed via one PE matmul vs identity -> invT [8, 128];
    per pair gpsimd partition_broadcast -> invrep [128, 128] sbuf
  - P2 = E * invrep on diag blocks only into persistent zero-padded
    [128,128] bf16 tiles (block-diagonal normalized-transposed probs)
  - q_b: PE transpose of the x tile pair-slice -> bf16 psum, copied to
    sbuf (DVE/ACT, split by schedule)
  - mm2: U[c, (j,b)] = qb^T-contracted with P2 (4 chunk matmuls N=128);
    by symmetry of E this is msgT with per-column normalization already
    applied; residual x is folded in as one extra identity-matmul
    accumulate per chunk for pairs whose combine runs on ACT
  - combine: U (+x) written into the x tile in place (DVE tensor_add or
    ACT copy), then one DMA per block writes the finished tile out
"""

import numpy as np

B, C, L_FULL, N_CORES = 64, 512, 4096, 8
LS = L_FULL // N_CORES  # 512 locations per core
LB = 16                 # locations per block (= DMA chunk)
N_BLK = LS // LB        # 32
N_PAIR = LB // 2        # 8 pairs per block
GRP = 8                 # pairs per group (reciprocal batch)
N_GRP = N_PAIR // GRP   # 1 group per block
CCH = C // 128          # 4 c-chunks
SCALE = 1.0 / float(C) ** 0.5

# engine schedule tuning (per 2-pair move unit, global counter)
COMB_ACT_NUM, COMB_ACT_DEN = 7, 20   # fraction of combines on ACT (+PE residual)
QB_ACT_NUM, QB_ACT_DEN = 1, 4        # fraction of qb copies on ACT
INTERLEAVE = False                   # interleave C(g-1) pairs into A(g)
SCORES_PACK = 1                      # pairs sharing one scores psum tile
XT_BUFS = 6
S_BUFS = 3                           # scores psum bufs
U_PAIRS = 1                          # pairs per U psum tile (1 or 2)
U_BUFS = 3


def build_nc():
    from contextlib import ExitStack

    import concourse.bass as bass
    import concourse.mybir as mybir
    from concourse.masks import make_identity
    from concourse.tile import TileContext

    f32 = mybir.dt.float32
    bf16 = mybir.dt.bfloat16
    AF = mybir.ActivationFunctionType

    nc = bass.Bass()
    x = nc.dram_tensor("x", [N_BLK, 128, CCH, B, LB], bf16, kind="ExternalInput")
    y = nc.dram_tensor("y", [N_BLK, 128, CCH, B, LB], bf16, kind="ExternalOutput")

    with ExitStack() as ctx:
        tc = ctx.enter_context(TileContext(nc))
        const = ctx.enter_context(tc.tile_pool(name="const", bufs=1))
        xt_pool = ctx.enter_context(tc.tile_pool(name="xt", bufs=XT_BUFS))
        sm = ctx.enter_context(tc.tile_pool(name="sm", bufs=2))
        ps = ctx.enter_context(tc.tile_pool(name="ps", bufs=2, space="PSUM"))

        ident = const.tile([128, 128], bf16, name="ident", tag="ident")
        make_identity(nc, ident)
        # persistent block-diagonal prob tiles: off-diagonal zeroed once
        p2t = []
        for i in range(4):
            t = const.tile([128, 128], bf16, name=f"p2_{i}", tag=f"p2_{i}")
            nc.vector.memset(t, 0.0)
            p2t.append(t)

        xt_tiles = {}
        grp_state = {}  # g -> dict with e tiles, qbs tiles, invTs, block
        qctr = [0]      # global 2-pair counter (combine schedule)
        qbctr = [0]     # global 2-pair counter (qb copy schedule)
        p2ctr = [0]

        def emit_A_pair(g, gp, st):
            b = g // N_GRP
            xt = xt_tiles[b]
            p = (g % N_GRP) * GRP + gp  # pair index within block
            if SCORES_PACK > 1:
                if gp % SCORES_PACK == 0:
                    st["ps_s"] = ps.tile(
                        [128, 64 * SCORES_PACK], f32,
                        name=f"s{g}_{gp}", tag="s", bufs=2,
                    )
                sp = gp % SCORES_PACK
                ps_s = st["ps_s"][:, sp * 64 : (sp + 1) * 64]
            else:
                ps_s = ps.tile([128, 64], f32, name=f"s{g}_{gp}", tag="s", bufs=S_BUFS)
            for j in (0, 1):
                for ci in range(CCH):
                    nc.tensor.matmul(
                        ps_s[j * 64 : (j + 1) * 64, :],
                        xt[:, ci, :, 2 * p + j],
                        xt[:, ci, :, 2 * p + j],
                        start=(ci == 0),
                        stop=(ci == CCH - 1),
                    )
            e = sm.tile([128, 64], bf16, name=f"e{g}_{gp}", tag="e", bufs=18)
            nc.scalar.activation(
                e,
                ps_s,
                AF.Exp,
                scale=SCALE,
                accum_out=st["invB"][:, gp : gp + 1],
            )
            st["e"].append(e)

            slot = gp % 2
            if slot == 0:
                st["qbps"] = ps.tile(
                    [128, 1024], bf16, name=f"qbp{g}_{gp}", tag="qbps", bufs=2
                )
            qbps = st["qbps"]
            for ci in range(CCH):
                nc.tensor.transpose(
                    qbps[:, slot * 512 + ci * 128 : slot * 512 + (ci + 1) * 128],
                    xt[:, ci, :, 2 * p : 2 * p + 2].rearrange("c b l -> c l b"),
                    ident,
                )
            if slot == 1:
                qbs = sm.tile(
                    [128, 1024], bf16, name=f"qbs{g}_{gp}", tag="qbs", bufs=8
                )
                on_act = (qbctr[0] * QB_ACT_NUM) % QB_ACT_DEN < QB_ACT_NUM
                qbctr[0] += 1
                if on_act:
                    nc.scalar.copy(qbs, qbps)
                else:
                    nc.vector.tensor_copy(qbs, qbps)
                st["qbs"].append(qbs)

        def emit_B(g):
            st = grp_state[g]
            invb = sm.tile([128, GRP], bf16, name=f"invb{g}", tag="invb", bufs=2)
            with nc.allow_low_precision(reason="softmax norm tolerates bf16"):
                nc.vector.reciprocal(invb, st["invB"])
            ps_t = ps.tile([GRP, 128], f32, name=f"it{g}", tag="s", bufs=S_BUFS)
            nc.tensor.matmul(ps_t, invb, ident, start=True, stop=True)
            invTs = sm.tile([GRP, 128], bf16, name=f"iT{g}", tag="invTs", bufs=2)
            nc.vector.tensor_copy(invTs, ps_t)
            st["invTs"] = invTs

        def emit_C_pair(g, gp):
            st = grp_state[g]
            b = g // N_GRP
            xt = xt_tiles[b]
            invTs = st["invTs"]
            p = (g % N_GRP) * GRP + gp
            invrep = sm.tile(
                [128, 128], bf16, name=f"ir{g}_{gp}", tag="invrep", bufs=3
            )
            nc.gpsimd.partition_broadcast(invrep, invTs[gp : gp + 1, :])
            e = st["e"][gp]
            P2 = p2t[p2ctr[0] % 4]
            p2ctr[0] += 1
            nc.gpsimd.tensor_mul(P2[0:64, 0:64], e[0:64, :], invrep[0:64, 0:64])
            nc.gpsimd.tensor_mul(
                P2[64:128, 64:128], e[64:128, :], invrep[64:128, 64:128]
            )

            slot = gp % U_PAIRS
            if slot == 0:
                st["U"] = ps.tile(
                    [128, 512 * U_PAIRS], f32, name=f"u{g}_{gp}", tag="u",
                    bufs=U_BUFS,
                )
                st["use_act"] = (
                    qctr[0] * COMB_ACT_NUM
                ) % COMB_ACT_DEN < COMB_ACT_NUM
                qctr[0] += 1
            U, use_act = st["U"], st["use_act"]
            qbs2 = st["qbs"][gp // 2]
            qsl0 = (gp % 2) * 512
            for ci in range(CCH):
                sl = slice(slot * 512 + ci * 128, slot * 512 + (ci + 1) * 128)
                qsl = slice(qsl0 + ci * 128, qsl0 + (ci + 1) * 128)
                nc.tensor.matmul(
                    U[:, sl], qbs2[:, qsl], P2, start=True, stop=not use_act
                )
                if use_act:
                    # residual: U += x^T (same layout), so ACT combine is a copy
                    nc.tensor.matmul(
                        U[:, sl], qbs2[:, qsl], ident, start=False, stop=True
                    )
            if slot == U_PAIRS - 1:
                pp = p - U_PAIRS + 1  # first pair of this U tile
                dst = xt[:, :, :, 2 * pp : 2 * pp + 2 * U_PAIRS].rearrange(
                    "c ci b (s j) -> c ci b s j", s=U_PAIRS
                )
                src = U.rearrange(
                    "c (s ci j b) -> c ci b s j", s=U_PAIRS, ci=CCH, j=2
                )
                if use_act:
                    nc.scalar.copy(dst, src)
                else:
                    nc.vector.tensor_add(dst, src, dst)

        def emit_group(g):
            # A(g) [optionally interleaved pairwise with C(g-1)], then B(g)
            st = {
                "e": [],
                "qbs": [],
                "invB": sm.tile([128, GRP], f32, name=f"invB{g}", tag="invB", bufs=2),
            }
            grp_state[g] = st
            if INTERLEAVE:
                for gp in range(GRP):
                    emit_A_pair(g, gp, st)
                    if g >= 1:
                        emit_C_pair(g - 1, gp)
            else:
                for gp in range(GRP):
                    emit_A_pair(g, gp, st)
                if g >= 1:
                    for gp in range(GRP):
                        emit_C_pair(g - 1, gp)
            emit_B(g)
            if g >= 1:
                grp_state.pop(g - 1)

        G_TOT = N_BLK * N_GRP
        for g in range(G_TOT):
            b = g // N_GRP
            if g % N_GRP == 0:
                xt = xt_pool.tile([128, CCH, B, LB], bf16, name=f"xt{b}", tag="xt")
                nc.sync.dma_start(out=xt, in_=x[b])
                xt_tiles[b] = xt
            emit_group(g)
            if g >= 1 and (g - 1) % N_GRP == N_GRP - 1:
                bb = (g - 1) // N_GRP
                nc.sync.dma_start(out=y[bb], in_=xt_tiles[bb])
        for gp in range(GRP):
            emit_C_pair(G_TOT - 1, gp)
        nc.sync.dma_start(out=y[N_BLK - 1], in_=xt_tiles[N_BLK - 1])

    _hoist_extra_waits(nc)
    return nc


def _hoist_extra_waits(nc):
    """The 64B instruction encodings have room for only one embedded
    sem-wait, but Tile sometimes emits 2+ (foreign engine + self).  Splice
    same-engine NoOps (one wait each) before such instructions; the
    instruction keeps its last wait plus its sem updates."""
    import concourse.mybir as mybir

    n_fixed = 0
    for f in nc.m.functions:
        for blk in f.blocks:
            new_insts = []
            for inst in blk.instructions:
                si = inst.sync_info
                if si is not None and len(si.on_wait) > 1:
                    waits = list(si.on_wait)
                    for wi, w in enumerate(waits[:-1]):
                        nop = mybir.InstNoOp(
                            name=f"{inst.name}-wsp{wi}", ins=[], outs=[]
                        )
                        nop.engine = inst.engine
                        nop.sync_info = mybir.SyncInfo(on_wait=[w], on_update=[])
                        new_insts.append(nop)
                    inst.sync_info = mybir.SyncInfo(
                        on_wait=[waits[-1]], on_update=list(si.on_update)
                    )
                    n_fixed += 1
                new_insts.append(inst)
            if n_fixed:
                try:
                    blk.instructions = new_insts
                except Exception:
                    blk.instructions.clear()
                    blk.instructions.extend(new_insts)
    return n_fixed


_NC_CACHE = {}


def _pack(xc: np.ndarray):
    """[B, C, LS] f32 -> [N_BLK, 128, CCH, B, LB] bf16 (contiguous)."""
    import ml_dtypes

    xr = xc.reshape(B, CCH, 128, N_BLK, LB).transpose(3, 2, 1, 0, 4)
    return np.ascontiguousarray(xr).astype(ml_dtypes.bfloat16)


def _unpack(yb: np.ndarray):
    """[N_BLK, 128, CCH, B, LB] bf16 -> [B, C, LS] f32."""
    return (
        yb.astype(np.float32).transpose(3, 2, 1, 0, 4).reshape(B, C, LS)
    )


def kernel(x: np.ndarray) -> np.ndarray:
    from concourse.bass_utils import run_bass_kernel_spmd

    assert x.shape == (B, C, L_FULL) and x.dtype == np.float32
    if "nc" not in _NC_CACHE:
        _NC_CACHE["nc"] = build_nc()
    nc = _NC_CACHE["nc"]

    in_maps = [
        {"x": _pack(x[:, :, i * LS : (i + 1) * LS])} for i in range(N_CORES)
    ]
    res = run_bass_kernel_spmd(nc, in_maps, core_ids=list(range(N_CORES)))
    out = np.concatenate(
        [_unpack(res.results[i]["y"]) for i in range(N_CORES)], axis=2
    )
    return out


# revision 19
# speedup vs baseline: 1.7511x; 1.1649x over previous
"""AttentionalPropagation kernel for Trainium2 (Bass/Tile), 8-core SPMD.

x: [B=64, C=512, L=4096] f32.  Per location l: self-attention over the B axis
(q=k=v, head dim C), out = x + msg.  Sharded over L: each core takes LS=512
locations; the host repacks its slice to bf16 in a per-block DMA-friendly
layout [blk, c(128), ci(4), l(16), b(64)] so every DMA moves 32KB-contiguous
runs per partition (full-bandwidth descriptors), and unpacks the same layout
on the way back.

Per-core dataflow (pairs of 2 locations, groups of 8 pairs):
  - mm1: scores compact [128=(j,b), 64=b'] psum, 8 matmuls of N=64 (K=c
    chunks of 128, two locations stacked on output partitions)
  - exp on ACT with accum_out giving rowsums S per partition; batched
    reciprocal (DVE) -> inv bf16 [128, 8] per group
  - inv transposed via one PE matmul vs identity -> invT [8, 128];
    per pair gpsimd partition_broadcast -> invrep [128, 128] sbuf
  - P2 = E * invrep on diag blocks only into persistent zero-padded
    [128,128] bf16 tiles (block-diagonal normalized-transposed probs)
  - q_b: PE transpose of the x tile pair-slice -> bf16 psum, copied to
    sbuf (DVE/ACT, split by schedule)
  - mm2: U[c, (j,b)] = qb^T-contracted with P2 (4 chunk matmuls N=128);
    by symmetry of E this is msgT with per-column normalization already
    applied; residual x is folded in as one extra identity-matmul
    accumulate per chunk for pairs whose combine runs on ACT
  - combine: U (+x) written into the x tile in place (DVE tensor_add or
    ACT copy), then one DMA per block writes the finished tile out
"""

import numpy as np

B, C, L_FULL, N_CORES = 64, 512, 4096, 8
LS = L_FULL // N_CORES  # 512 locations per core
LB = 16                 # locations per block (= DMA chunk)
N_BLK = LS // LB        # 32
N_PAIR = LB // 2        # 8 pairs per block
GRP = 8                 # pairs per group (reciprocal batch)
N_GRP = N_PAIR // GRP   # 1 group per block
CCH = C // 128          # 4 c-chunks
SCALE = 1.0 / float(C) ** 0.5

# engine schedule tuning (per 2-pair move unit, global counter)
COMB_ACT_NUM, COMB_ACT_DEN = 7, 20   # fraction of combines on ACT (+PE residual)
QB_ACT_NUM, QB_ACT_DEN = 1, 4        # fraction of qb copies on ACT
INTERLEAVE = False                   # interleave C(g-1) pairs into A(g)
SCORES_PACK = 1                      # pairs sharing one scores psum tile
XT_BUFS = 7
S_BUFS = 3                           # scores psum bufs
U_PAIRS = 1                          # pairs per U psum tile (1 or 2)
U_BUFS = 3
OUT_DELAY = 2                        # blocks between compute and out-DMA emission
C_LAG = 2                            # groups between A(g) and C(g-C_LAG)
N_P2 = 8                             # rotating persistent P2 tiles
C_FIRST = False                      # emit C batch before A batch


def build_nc():
    from contextlib import ExitStack

    import concourse.bass as bass
    import concourse.mybir as mybir
    from concourse.masks import make_identity
    from concourse.tile import TileContext

    f32 = mybir.dt.float32
    bf16 = mybir.dt.bfloat16
    AF = mybir.ActivationFunctionType

    nc = bass.Bass()
    x = nc.dram_tensor("x", [N_BLK, 128, CCH, LB, B], bf16, kind="ExternalInput")
    y = nc.dram_tensor("y", [N_BLK, 128, CCH, LB, B], bf16, kind="ExternalOutput")

    with ExitStack() as ctx:
        tc = ctx.enter_context(TileContext(nc))
        const = ctx.enter_context(tc.tile_pool(name="const", bufs=1))
        xt_pool = ctx.enter_context(tc.tile_pool(name="xt", bufs=XT_BUFS))
        sm = ctx.enter_context(tc.tile_pool(name="sm", bufs=2))
        ps = ctx.enter_context(tc.tile_pool(name="ps", bufs=2, space="PSUM"))

        ident = const.tile([128, 128], bf16, name="ident", tag="ident")
        make_identity(nc, ident)
        # persistent block-diagonal prob tiles: off-diagonal zeroed once
        p2t = []
        for i in range(N_P2):
            t = const.tile([128, 128], bf16, name=f"p2_{i}", tag=f"p2_{i}")
            nc.vector.memset(t, 0.0)
            p2t.append(t)

        xt_tiles = {}
        grp_state = {}  # g -> dict with e tiles, qbs tiles, invTs, block
        qctr = [0]      # global 2-pair counter (combine schedule)
        qbctr = [0]     # global 2-pair counter (qb copy schedule)
        p2ctr = [0]

        def emit_A_pair(g, gp, st):
            b = g // N_GRP
            xt = xt_tiles[b]
            p = (g % N_GRP) * GRP + gp  # pair index within block
            if SCORES_PACK > 1:
                if gp % SCORES_PACK == 0:
                    st["ps_s"] = ps.tile(
                        [128, 64 * SCORES_PACK], f32,
                        name=f"s{g}_{gp}", tag="s", bufs=2,
                    )
                sp = gp % SCORES_PACK
                ps_s = st["ps_s"][:, sp * 64 : (sp + 1) * 64]
            else:
                ps_s = ps.tile([128, 64], f32, name=f"s{g}_{gp}", tag="s", bufs=S_BUFS)
            for j in (0, 1):
                for ci in range(CCH):
                    nc.tensor.matmul(
                        ps_s[j * 64 : (j + 1) * 64, :],
                        xt[:, ci, 2 * p + j, :],
                        xt[:, ci, 2 * p + j, :],
                        start=(ci == 0),
                        stop=(ci == CCH - 1),
                    )
            e = sm.tile([128, 64], bf16, name=f"e{g}_{gp}", tag="e", bufs=8 * (C_LAG + 1) + 2)
            nc.scalar.activation(
                e,
                ps_s,
                AF.Exp,
                scale=SCALE,
                accum_out=st["invB"][:, gp : gp + 1],
            )
            st["e"].append(e)

            slot = gp % 2
            if slot == 0:
                st["qbps"] = ps.tile(
                    [128, 1024], bf16, name=f"qbp{g}_{gp}", tag="qbps", bufs=2
                )
            qbps = st["qbps"]
            for ci in range(CCH):
                nc.tensor.transpose(
                    qbps[:, slot * 512 + ci * 128 : slot * 512 + (ci + 1) * 128],
                    xt[:, ci, 2 * p : 2 * p + 2, :],
                    ident,
                )
            if slot == 1:
                qbs = sm.tile(
                    [128, 1024], bf16, name=f"qbs{g}_{gp}", tag="qbs", bufs=4 * (C_LAG + 1) + 1
                )
                on_act = (qbctr[0] * QB_ACT_NUM) % QB_ACT_DEN < QB_ACT_NUM
                qbctr[0] += 1
                if on_act:
                    nc.scalar.copy(qbs, qbps)
                else:
                    nc.vector.tensor_copy(qbs, qbps)
                st["qbs"].append(qbs)

        def emit_B(g):
            st = grp_state[g]
            invb = sm.tile([128, GRP], bf16, name=f"invb{g}", tag="invb", bufs=C_LAG + 1)
            with nc.allow_low_precision(reason="softmax norm tolerates bf16"):
                nc.vector.reciprocal(invb, st["invB"])
            ps_t = ps.tile([GRP, 128], f32, name=f"it{g}", tag="s", bufs=S_BUFS)
            nc.tensor.matmul(ps_t, invb, ident, start=True, stop=True)
            invTs = sm.tile([GRP, 128], bf16, name=f"iT{g}", tag="invTs", bufs=C_LAG + 1)
            nc.vector.tensor_copy(invTs, ps_t)
            st["invTs"] = invTs

        def emit_C_pair(g, gp):
            st = grp_state[g]
            b = g // N_GRP
            xt = xt_tiles[b]
            invTs = st["invTs"]
            p = (g % N_GRP) * GRP + gp
            invrep = sm.tile(
                [128, 128], bf16, name=f"ir{g}_{gp}", tag="invrep", bufs=3
            )
            nc.gpsimd.partition_broadcast(invrep, invTs[gp : gp + 1, :])
            e = st["e"][gp]
            P2 = p2t[p2ctr[0] % N_P2]
            p2ctr[0] += 1
            nc.gpsimd.tensor_mul(P2[0:64, 0:64], e[0:64, :], invrep[0:64, 0:64])
            nc.gpsimd.tensor_mul(
                P2[64:128, 64:128], e[64:128, :], invrep[64:128, 64:128]
            )

            slot = gp % U_PAIRS
            if slot == 0:
                st["U"] = ps.tile(
                    [128, 512 * U_PAIRS], f32, name=f"u{g}_{gp}", tag="u",
                    bufs=U_BUFS,
                )
                st["use_act"] = (
                    qctr[0] * COMB_ACT_NUM
                ) % COMB_ACT_DEN < COMB_ACT_NUM
                qctr[0] += 1
            U, use_act = st["U"], st["use_act"]
            qbs2 = st["qbs"][gp // 2]
            qsl0 = (gp % 2) * 512
            for ci in range(CCH):
                sl = slice(slot * 512 + ci * 128, slot * 512 + (ci + 1) * 128)
                qsl = slice(qsl0 + ci * 128, qsl0 + (ci + 1) * 128)
                nc.tensor.matmul(
                    U[:, sl], qbs2[:, qsl], P2, start=True, stop=not use_act
                )
                if use_act:
                    # residual: U += x^T (same layout), so ACT combine is a copy
                    nc.tensor.matmul(
                        U[:, sl], qbs2[:, qsl], ident, start=False, stop=True
                    )
            if slot == U_PAIRS - 1:
                pp = p - U_PAIRS + 1  # first pair of this U tile
                dst = xt[:, :, 2 * pp : 2 * pp + 2 * U_PAIRS, :].rearrange(
                    "c ci (s j) b -> c ci s j b", s=U_PAIRS
                )
                src = U.rearrange(
                    "c (s ci j b) -> c ci s j b", s=U_PAIRS, ci=CCH, j=2
                )
                if use_act:
                    nc.scalar.copy(dst, src)
                else:
                    nc.vector.tensor_add(dst, src, dst)

        def emit_group(g):
            # A(g) [optionally interleaved pairwise with C(g-1)], then B(g)
            st = {
                "e": [],
                "qbs": [],
                "invB": sm.tile([128, GRP], f32, name=f"invB{g}", tag="invB", bufs=C_LAG + 1),
            }
            grp_state[g] = st
            gc = g - C_LAG
            if INTERLEAVE:
                for gp in range(GRP):
                    emit_A_pair(g, gp, st)
                    if gc >= 0:
                        emit_C_pair(gc, gp)
            elif C_FIRST and gc >= 0:
                for gp in range(GRP):
                    emit_C_pair(gc, gp)
                for gp in range(GRP):
                    emit_A_pair(g, gp, st)
            else:
                for gp in range(GRP):
                    emit_A_pair(g, gp, st)
                if gc >= 0:
                    for gp in range(GRP):
                        emit_C_pair(gc, gp)
            emit_B(g)
            if gc >= 0:
                grp_state.pop(gc)

        G_TOT = N_BLK * N_GRP
        for g in range(G_TOT):
            b = g // N_GRP
            if g % N_GRP == 0:
                xt = xt_pool.tile([128, CCH, LB, B], bf16, name=f"xt{b}", tag="xt")
                nc.sync.dma_start(out=xt, in_=x[b])
                xt_tiles[b] = xt
            emit_group(g)
            gd = g - OUT_DELAY - (C_LAG - 1)
            if gd >= 0 and gd % N_GRP == N_GRP - 1:
                bb = gd // N_GRP
                nc.sync.dma_start(out=y[bb], in_=xt_tiles[bb])
        for gc in range(G_TOT - C_LAG, G_TOT):
            for gp in range(GRP):
                emit_C_pair(gc, gp)
        for gd in range(G_TOT - OUT_DELAY - (C_LAG - 1), G_TOT):
            if gd >= 0 and gd % N_GRP == N_GRP - 1:
                nc.sync.dma_start(out=y[gd // N_GRP], in_=xt_tiles[gd // N_GRP])

    _hoist_extra_waits(nc)
    return nc


def _hoist_extra_waits(nc):
    """The 64B instruction encodings have room for only one embedded
    sem-wait, but Tile sometimes emits 2+ (foreign engine + self).  Splice
    same-engine NoOps (one wait each) before such instructions; the
    instruction keeps its last wait plus its sem updates."""
    import concourse.mybir as mybir

    n_fixed = 0
    for f in nc.m.functions:
        for blk in f.blocks:
            new_insts = []
            for inst in blk.instructions:
                si = inst.sync_info
                if si is not None and len(si.on_wait) > 1:
                    waits = list(si.on_wait)
                    for wi, w in enumerate(waits[:-1]):
                        nop = mybir.InstNoOp(
                            name=f"{inst.name}-wsp{wi}", ins=[], outs=[]
                        )
                        nop.engine = inst.engine
                        nop.sync_info = mybir.SyncInfo(on_wait=[w], on_update=[])
                        new_insts.append(nop)
                    inst.sync_info = mybir.SyncInfo(
                        on_wait=[waits[-1]], on_update=list(si.on_update)
                    )
                    n_fixed += 1
                new_insts.append(inst)
            if n_fixed:
                try:
                    blk.instructions = new_insts
                except Exception:
                    blk.instructions.clear()
                    blk.instructions.extend(new_insts)
    return n_fixed


_NC_CACHE = {}


def _pack(xc: np.ndarray):
    """[B, C, LS] f32 -> [N_BLK, 128, CCH, B, LB] bf16 (contiguous)."""
    import ml_dtypes

    xr = xc.reshape(B, CCH, 128, N_BLK, LB).transpose(3, 2, 1, 4, 0)
    return np.ascontiguousarray(xr).astype(ml_dtypes.bfloat16)


def _unpack(yb: np.ndarray):
    """[N_BLK, 128, CCH, B, LB] bf16 -> [B, C, LS] f32."""
    return (
        yb.astype(np.float32).transpose(4, 2, 1, 0, 3).reshape(B, C, LS)
    )


def kernel(x: np.ndarray) -> np.ndarray:
    from concourse.bass_utils import run_bass_kernel_spmd

    assert x.shape == (B, C, L_FULL) and x.dtype == np.float32
    if "nc" not in _NC_CACHE:
        _NC_CACHE["nc"] = build_nc()
    nc = _NC_CACHE["nc"]

    in_maps = [
        {"x": _pack(x[:, :, i * LS : (i + 1) * LS])} for i in range(N_CORES)
    ]
    res = run_bass_kernel_spmd(nc, in_maps, core_ids=list(range(N_CORES)))
    out = np.concatenate(
        [_unpack(res.results[i]["y"]) for i in range(N_CORES)], axis=2
    )
    return out
